# revision 34
# baseline (speedup 1.0000x reference)
"""CameraAwareMemory loss kernel for 8 Trainium2 NeuronCores.

Strategy: camera-sharding — core k owns ALL 4096 proxies of camera k
(P=32768, C=8).  Everything feeds from fp8(e4m3) DoubleRow matmuls (the
256-deep contraction rides two packed k-halves, one instruction per
512-col bank half):
  - sims' = (feat + r*mem[prx]) @ memT: single-fp8 matmul (selection
    noise only; see below)
  - score = feat @ memT: 3-way split-precision accumulation
    f8*m8 + phi8*m8 + f8*d8 where f8/phi8 and m8/d8 are the fp8 value +
    fp8-of-residual of feat and memT — ~3e-3 worst-case score error at
    2.4x less DMA and 25% less PE time than a bf16 matmul pair.
Then, over 1024-col double-bank PSUM tiles:
  - camera sum of exp(score/TEMP - mhat): fused exp+accumulate on the
    scalar engine, in place on PSUM (5 partial sums per row-tile, the
    first 1024-col tile split in two so the act stream starts early;
    host adds them).  mhat is a host-computed per-row bias, identical on
    all cores, statistically pinned to the row max.
  - per-64-proxy-window max of sims' via one DVE windowed tensor_reduce
    per 1024-col tile, direct on PSUM (16 windows per tile; the window
    POSITION identifies the proxies, so no max_index pass is needed)
The host merges the 8 cores' partials into the exact loss: the union of
the top-J windows per row provably covers every proxy the reference's
top-k selections can touch (a window containing the k-th largest value
always ranks within the top-k windows by window-max), and the host
recomputes exact fp32 scores at the candidate proxies so no selection
decision depends on fp8 rounding (fp8 only perturbs WHICH windows are
expanded; margins JG/DELTA absorb its ~0.05 value noise).
"""

import sys

import numpy as np

sys.path.insert(0, "/opt/trn_rl_repo")

# ---- problem constants (hardcoded per spec) ----
P = 32768
D = 256
C = 8
B = 256
TEMP = 0.05
BG_KNN = 50
POSK = 3
BAL_W = 0.15
RATIO = (1.0 - BAL_W) / BAL_W        # 5.666...: sims' = score + RATIO*q  (same order as sims)
INV_TEMP = 1.0 / TEMP                # 20.0
NCORES = 8
PSH = P // NCORES                    # 4096 proxies per core (= one camera)
WIN = 64                             # proxies per candidate window
NWIN_CORE = PSH // WIN               # 64 windows per core
NT = 4                               # 1024-col tiles per row-tile
TW = PSH // NT                       # 1024 cols per tile
# score/activation subtiles: the act stream is saturated, so its TOTAL
# length sets the wall.  Row-tile 0 leads the stream, so its first 1024-col
# tile is split in two (the stream starts as soon as one 512-col half has
# arrived); row-tile 1's subtiles are never start-gating, so it uses four
# 1024-col acts (less per-instruction overhead).  rt1's unused 5th partial-
# sum column is memset to zero.
ACTS_RT = [
    [(0, 512), (512, 1024), (1024, 2048), (2048, 3072), (3072, 4096)],
    [(0, 1024), (1024, 2048), (2048, 3072), (3072, 4096)],
]
NACT = len(ACTS_RT[0])               # 5 partial-sum columns in the output
OBW = NWIN_CORE + NACT               # out cols per row-tile: 64 maxes + 5 partial sums

_CACHE = {}


def _build_bass():
    import concourse.bacc as bacc
    import concourse.mybir as mybir
    import concourse.tile as tile
    from contextlib import ExitStack

    f32 = mybir.dt.float32
    bf16 = mybir.dt.bfloat16
    f8 = mybir.dt.float8e4
    AF = mybir.ActivationFunctionType

    nc = bacc.Bacc("TRN2", target_bir_lowering=False, debug=False)

    # fp8 pack: [f8(feat) | phi8(feat residual) | sims lhsT | m8(memT)] per
    # k-half.  The score matmul runs as a 3-way fp8 split (f8*m8 + phi8*m8 +
    # f8*d8, error ~3e-3 on unit-scale scores) so the whole kernel feeds from
    # fp8 — no bf16 pack at all, and m8 is shared with the sims matmul.
    PK8 = 3 * 256 + PSH                   # 4864
    pack8_d = nc.dram_tensor("pack8", [D, PK8], f8, kind="ExternalInput")
    packd_d = nc.dram_tensor("packd", [D, PSH], f8, kind="ExternalInput")
    nbias_d = nc.dram_tensor("nbias", [B, 1], f32, kind="ExternalInput")
    ob_d = nc.dram_tensor("ob", [B, OBW], f32, kind="ExternalOutput")

    with tile.TileContext(nc) as tc, ExitStack() as ctx:
        consts = ctx.enter_context(tc.tile_pool(name="consts", bufs=1))
        pqp = ctx.enter_context(tc.tile_pool(name="pq", bufs=2, space="PSUM"))
        psp = ctx.enter_context(tc.tile_pool(name="psc", bufs=2, space="PSUM"))
        small = ctx.enter_context(tc.tile_pool(name="small", bufs=2))
        outp = ctx.enter_context(tc.tile_pool(name="outp", bufs=2))

        pack8_sb = consts.tile([128, 2 * PK8], f8, tag="pack8")
        packd_sb = consts.tile([128, 2 * PSH], f8, tag="packd")
        p8_src = pack8_d.rearrange("(k p) c -> p k c", k=2)
        p8_dst = pack8_sb.rearrange("p (k c) -> p k c", k=2)
        pd_src = packd_d.rearrange("(k p) c -> p k c", k=2)
        pd_dst = packd_sb.rearrange("p (k c) -> p k c", k=2)

        # DMA order = first-use order: each tile consumes [m8 tile cols] then
        # [d8 tile cols] (the d8 matmul is last in the score accumulation)
        nc.sync.dma_start(out=p8_dst[:, :, 0:1280], in_=p8_src[:, :, 0:1280])
        nc.sync.dma_start(out=pd_dst[:, :, 0:512], in_=pd_src[:, :, 0:512])
        nc.sync.dma_start(out=p8_dst[:, :, 1280:1792], in_=p8_src[:, :, 1280:1792])
        nc.sync.dma_start(out=pd_dst[:, :, 512:1024], in_=pd_src[:, :, 512:1024])
        for g in range(1, 4):
            lo, hi = 768 + g * 1024, 768 + (g + 1) * 1024
            nc.sync.dma_start(out=p8_dst[:, :, lo:hi], in_=p8_src[:, :, lo:hi])
            nc.sync.dma_start(out=pd_dst[:, :, lo - 768:hi - 768],
                              in_=pd_src[:, :, lo - 768:hi - 768])

        # warm scratch init on the otherwise-idle gpsimd queue, BEFORE the
        # nbias DMAs so the PE warm-up isn't blocked behind them.  The warm
        # matmuls keep the PE busy from ~1us so the clock is ramped when the
        # real matmuls start; a dummy activation preloads the Exp table.
        warm_in = consts.tile([128, 656], bf16, tag="warm")
        nc.gpsimd.memset(warm_in[:], 0.0)
        # Exp-table preload reads a spare slice of the warm tile through a
        # bitcast (disjoint from the matmul operands so the slow table load
        # doesn't chain in front of the warm-up) — no second memset needed
        warm_f = warm_in[:, 640:656].bitcast(f32)
        nc.scalar.activation(warm_f, warm_f, AF.Exp)

        # per-row-tile -mhat bias (tiny; idle gpsimd queue, needed later)
        mbneg = []
        for rt in range(2):
            t = small.tile([128, 1], f32, tag=f"mbneg{rt}", name=f"mbneg_{rt}")
            nc.gpsimd.dma_start(out=t[:], in_=nbias_d[rt * 128:(rt + 1) * 128, :])
            mbneg.append(t)

        wl, wr = warm_in[:, 0:128], warm_in[:, 128:640]
        warm_ps = pqp.tile([128, 512], f32, tag="ps", name="warm_ps")
        for _ in range(4):
            nc.tensor.matmul(warm_ps[:], lhsT=wl, rhs=wr, start=True, stop=True)

        # one SBUF tile holds both row-tiles' outputs so each of the two
        # output DMAs is a single 3D-AP instruction (avoids SP.SEQ/HWDGE
        # serialization of four per-rt DMAs at the tail)
        ob_sb = outp.tile([128, 2 * OBW], f32, tag="ob", name="ob_sb")
        ob_t = [ob_sb[:, 0:OBW], ob_sb[:, OBW:2 * OBW]]
        # rt1 emits only 4 partial sums; zero its unused 5th column
        nc.gpsimd.memset(ob_sb[:, 2 * OBW - 1: 2 * OBW], 0.0)

        pack8_3d = pack8_sb.rearrange("p (k c) -> p k c", k=2)
        packd_3d = packd_sb.rearrange("p (k c) -> p k c", k=2)

        def w8_ap(sec, rt):                    # fp8 lhsT [128, 2, 128]
            base = sec * 256 + rt * 128        # sec: 0=f8, 1=phi8, 2=sims
            return pack8_3d[:, :, base: base + 128]

        def rhs8(lo, hi):                      # fp8 m8 cols [128, 2, w]
            return pack8_3d[:, :, 768 + lo: 768 + hi]

        def rhsd(lo, hi):                      # fp8 d8 cols [128, 2, w]
            return packd_3d[:, :, lo:hi]

        DR = mybir.MatmulPerfMode.DoubleRow

        for t in range(NT):
            lo = t * TW
            for rt in range(2):  # row-tile: batch rows [rt*128, rt*128+128)
                ps_q = pqp.tile([128, TW], f32, tag="ps", name=f"psq_{t}_{rt}")
                # fp8 DoubleRow: both k-halves in one pass, 512 cols per
                # instruction (fp8 moving-operand limit)
                for h in range(2):
                    nc.tensor.matmul(
                        ps_q[:, h * 512:(h + 1) * 512],
                        lhsT=w8_ap(2, rt),
                        rhs=rhs8(lo + h * 512, lo + (h + 1) * 512),
                        start=True, stop=True, perf_mode=DR,
                    )
                # window maxes of sims': one DVE scan over both banks.  The
                # stream-leading (t0, rt0) reduce is split per bank half so
                # the DVE stream starts ~0.8us earlier (it is saturated end
                # to end, so an earlier start drains it — and the windows
                # DMA it gates — correspondingly earlier).
                base = t * (TW // WIN)
                halves = 2 if (t == 0 and rt == 0) else 1
                hw_ = TW // halves
                for rh in range(halves):
                    nc.vector.tensor_reduce(
                        out=ob_t[rt][:, base + rh * (hw_ // WIN):
                                     base + (rh + 1) * (hw_ // WIN)],
                        in_=ps_q[:, rh * hw_:(rh + 1) * hw_].rearrange(
                            "p (w c) -> p w c", c=WIN),
                        axis=mybir.AxisListType.X,
                        op=mybir.AluOpType.max,
                    )
                # score via 3-way fp8 split accumulation per 512-col bank
                # half: f8*m8 + phi8*m8 + f8*d8 (the dropped phi8*d8 term is
                # ~1e-4); one fused in-place exp/accumulate per ACT subtile
                # (PSUM out beats the SBUF write port; only the accum_out
                # column is consumed)
                for a, (alo, ahi) in enumerate(ACTS_RT[rt]):
                    if not (alo >= lo and ahi <= lo + TW):
                        continue
                    ps_s = psp.tile([128, ahi - alo], f32, tag="ps",
                                    name=f"pss_{a}_{rt}")
                    for h in range((ahi - alo) // 512):
                        hl, hh = alo + h * 512, alo + (h + 1) * 512
                        parts = [(0, rhs8(hl, hh)), (1, rhs8(hl, hh)),
                                 (0, rhsd(hl, hh))]
                        for i, (sec, rhs) in enumerate(parts):
                            nc.tensor.matmul(
                                ps_s[:, h * 512:(h + 1) * 512],
                                lhsT=w8_ap(sec, rt), rhs=rhs,
                                start=(i == 0), stop=(i == len(parts) - 1),
                                perf_mode=DR,
                            )
                    nc.scalar.activation(
                        ps_s[:], ps_s[:], AF.Exp,
                        bias=mbneg[rt][:], scale=INV_TEMP,
                        accum_out=ob_t[rt][:, NWIN_CORE + a: NWIN_CORE + a + 1],
                    )

        # v8 windows ship as soon as the DVE stream drains; the tiny zpart
        # columns (gated by the later-finishing act stream) go separately so
        # the final transfer is minimal.  Both are single 3D-AP DMAs; the
        # windows ride the Pool SWDGE path so their descriptor prep doesn't
        # hold the HWDGE device in front of the final zpart DMA.
        ob_dst = ob_d.rearrange("(t p) c -> p t c", t=2)
        ob_src = ob_sb.rearrange("p (t c) -> p t c", t=2)
        nc.sync.dma_start(out=ob_dst[:, :, :NWIN_CORE],
                          in_=ob_src[:, :, :NWIN_CORE])
        nc.sync.dma_start(out=ob_dst[:, :, NWIN_CORE:],
                          in_=ob_src[:, :, NWIN_CORE:])

    nc.compile()
    return nc


def _get_nc():
    if "nc" not in _CACHE:
        _CACHE["nc"] = _build_bass()
    return _CACHE["nc"]


def _run_device(in_maps, trace=False):
    from concourse.bass_utils import run_bass_kernel_spmd

    nc = _get_nc()
    res = run_bass_kernel_spmd(
        nc, in_maps, core_ids=list(range(NCORES)), trace=trace
    )
    return res


def kernel(features, targets, cams, epoch, global_memory, all_pseudo_label,
           all_proxy_label, cam_proxies, label_proxies, _want_trace=False):
    import ml_dtypes

    feat = np.ascontiguousarray(np.asarray(features), dtype=np.float32)
    mem = np.ascontiguousarray(np.asarray(global_memory), dtype=np.float32)
    targets = np.asarray(targets).astype(np.int64)
    cams_h = np.asarray(cams).astype(np.int64)
    apl = np.asarray(all_proxy_label).astype(np.int64)
    apsl = np.asarray(all_pseudo_label).astype(np.int64)
    cam_prox = np.asarray(cam_proxies).astype(np.int64)   # [C, PSH]
    lab_prox = np.asarray(label_proxies).astype(np.int64)

    prx = apl[targets]                      # [B] target proxy
    pseudo_y = apsl[targets]                # [B]
    pos_cols = lab_prox[pseudo_y]           # [B, C] positive proxies (cross)
    memprx = mem[prx]                       # [B, D]

    # camera of each proxy; core k owns exactly camera k's proxies
    cam_of_p = np.empty(P, np.int64)
    cam_of_p[cam_prox.reshape(-1)] = np.repeat(np.arange(C), cam_prox.shape[1])
    assert cam_prox.shape == (C, PSH)

    memT = mem.T                            # [D, P]
    lhs2 = feat + np.float32(RATIO) * memprx           # [B, D] sims rows
    # Row-global exp bias: x = INV_TEMP * feat@mem_p with unit mem rows, so
    # x_row ~ N(0, (INV_TEMP*|feat|/sqrt(D))^2).  4.5 sigma sits within
    # [x_max - 80, x_max + ~25] for a 32768-sample max, so exp(x - mhat)
    # neither overflows nor flushes any term that matters.  Identical across
    # cores, so the merge is a plain sum.
    mhat = (4.5 * INV_TEMP / np.sqrt(D)) * np.linalg.norm(
        feat.astype(np.float64), axis=1)    # [B]
    nbias = np.ascontiguousarray((-mhat[:, None]).astype(np.float32))
    f8dt = ml_dtypes.float8_e4m3
    featT = np.ascontiguousarray(feat.T)                # [D, B]
    f8 = featT.astype(f8dt)                             # fp8(feat)
    phi8 = (featT - f8.astype(np.float32)).astype(f8dt)  # fp8 residual
    s8 = lhs2.T.astype(f8dt)                            # sims lhsT
    in_maps = []
    for k in range(NCORES):
        shard = np.ascontiguousarray(memT[:, cam_prox[k]])  # [D, 4096]
        m8 = shard.astype(f8dt)
        d8 = (shard - m8.astype(np.float32)).astype(f8dt)
        pack8 = np.hstack([f8, phi8, s8, m8])           # [D, 4864]
        in_maps.append({
            "pack8": np.ascontiguousarray(pack8),
            "packd": np.ascontiguousarray(d8),
            "nbias": nbias,
        })

    res = _run_device(in_maps, trace=_want_trace)
    results = res.results
    if _want_trace:
        _CACHE["last_exec_time_ns"] = res.exec_time_ns

    ob = np.stack([r["ob"] for r in results]).astype(np.float64)  # [K, B, OBW]
    zpart = ob[:, :, NWIN_CORE:]                                  # [K, B, NT]
    v8 = ob[:, :, :NWIN_CORE]                                     # [K, B, 64]

    rows = np.arange(B)

    # ---- logsumexp merge (cross / intra) ----
    mhat_used = -nbias[:, 0].astype(np.float64)               # exact bias device used
    Zc = zpart.sum(axis=2).T                                  # [B, C] (core k = cam k)
    lse_full = mhat_used + np.log(Zc.sum(axis=1))             # logsumexp over all P of x
    lse_cam = mhat_used + np.log(Zc[rows, cams_h])            # over own camera's proxies

    x_prx = INV_TEMP * np.einsum("bd,bd->b", feat.astype(np.float64),
                                 memprx.astype(np.float64))
    # If a sample's camera does not own its target proxy (possible when cams
    # is generated independently of targets), the reference's one-hot mask is
    # all-zero and its intra term is exactly 0.
    present = cam_of_p[prx] == cams_h
    intra = np.where(present, lse_cam - x_prx, 0.0)

    x_pos = INV_TEMP * np.einsum("bd,bkd->bk", feat.astype(np.float64),
                                 mem[pos_cols].astype(np.float64))
    cross = lse_full - x_pos.mean(axis=1)

    # ---- online loss ----
    # v8[k, b, w] = fp8-accurate max of sims' over window w of core/camera k
    # (proxies cam_prox[k, w*64 .. +64]).  Select candidate windows per row:
    # the global top windows (covers the reference's top-(BG_KNN+POSK)
    # proxies: the window holding the k-th largest value always ranks within
    # the top-k windows) plus every window within DELTA of its camera's best
    # (covers per-camera argmax).  Expand the selected windows and recompute
    # exact fp32 sims'/x there.  Margins sized for fp8 matmul noise
    # (sigma ~0.05 on window maxes).
    W = NCORES * NWIN_CORE                                    # 512 windows/row
    wv = np.moveaxis(v8, 0, 1).reshape(B, W)                  # [B, 512] k-major
    cam_of_w = np.repeat(np.arange(C), NWIN_CORE)             # [512]
    DELTA = 0.4
    JG = 88                                                   # global windows
    cammax = wv.reshape(B, C, NWIN_CORE).max(axis=2)          # [B, C]
    boost = wv >= (cammax[:, cam_of_w] - DELTA)               # near-camera-top
    nboost = int(boost.sum(axis=1).max())
    J = JG + max(nboost, C)
    prio = wv + 1e9 * boost
    sel_w = np.argpartition(-prio, J - 1, axis=1)[:, :J]      # [B, J] unique

    k_of = sel_w // NWIN_CORE                                 # camera/core
    w_of = sel_w % NWIN_CORE
    pid = cam_prox[k_of[:, :, None],
                   (w_of * WIN)[:, :, None] + np.arange(WIN)[None, None, :]]
    pid_b = pid.reshape(B, J * WIN)
    cam_of_cand = np.repeat(cam_of_w[sel_w], WIN, axis=1)     # [B, J*WIN]

    # exact fp32 recompute at the candidate proxies (row-chunked: the
    # gather is the memory hog)
    NCAND = J * WIN
    s_cand = np.empty((B, NCAND), np.float32)
    q_cand = np.empty((B, NCAND), np.float32)
    for lo in range(0, B, 32):
        hi = lo + 32
        memg = mem[pid_b[lo:hi]]                              # [32, NCAND, D]
        s_cand[lo:hi] = np.einsum("bd,bjd->bj", feat[lo:hi], memg)
        q_cand[lo:hi] = np.einsum("bd,bjd->bj", memprx[lo:hi], memg)
    simsp = s_cand.astype(np.float64) + RATIO * q_cand.astype(np.float64)
    x_cand = INV_TEMP * s_cand.astype(np.float64)

    # per-camera global argmax over candidates (exact values)
    tops_val = np.full((B, C), -np.inf)
    tops_j = np.zeros((B, C), np.int64)
    for c in range(C):
        sub = np.where(cam_of_cand == c, simsp, -np.inf)
        a = sub.argmax(axis=1)
        tops_j[:, c] = a
        tops_val[:, c] = sub[rows, a]

    # top-3 cameras by their best sims'
    order = np.argsort(-tops_val, axis=1)[:, :POSK]           # [B, 3]
    chosen_j = np.take_along_axis(tops_j, order, axis=1)      # [B, 3] cand idx
    chosen_pid = np.take_along_axis(pid_b, chosen_j, axis=1)  # [B, 3]

    # top-50 of the remaining candidates (windows are disjoint, so every
    # candidate proxy appears once; only the chosen need masking)
    is_chosen = (pid_b[:, :, None] == chosen_pid[:, None, :]).any(axis=2)
    Vmask = np.where(is_chosen, -np.inf, simsp)
    sel_idx = np.argpartition(-Vmask, BG_KNN, axis=1)[:, :BG_KNN]     # [B, 50]

    x_chosen = np.take_along_axis(x_cand, chosen_j, axis=1)   # [B, 3]
    x_sel = np.take_along_axis(x_cand, sel_idx, axis=1)       # [B, 50]
    xA = np.concatenate([x_chosen, x_sel], axis=1)            # [B, 53]
    mA = xA.max(axis=1)
    lse3 = mA + np.log(np.exp(xA - mA[:, None]).sum(axis=1))
    online = lse3 - x_chosen.mean(axis=1)

    # ---- camera-mean-sum ----
    dbg = globals().get("_DEBUG_COMPS")
    if dbg is not None:
        dbg["intra"] = intra.copy()
        dbg["cross"] = cross.copy()
        dbg["online"] = online.copy()
    total = 0.0
    for c in range(C):
        m = cams_h == c
        if m.any():
            total += intra[m].mean() + cross[m].mean() + online[m].mean()
    return np.float32(total)


# revision 35
# speedup vs baseline: 1.0105x; 1.0105x over previous
"""CameraAwareMemory loss kernel for 8 Trainium2 NeuronCores.

Strategy: camera-sharding — core k owns ALL 4096 proxies of camera k
(P=32768, C=8).  Everything feeds from fp8(e4m3) DoubleRow matmuls (the
256-deep contraction rides two packed k-halves, one instruction per
512-col bank half):
  - sims' = (feat + r*mem[prx]) @ memT: single-fp8 matmul (selection
    noise only; see below)
  - score = feat @ memT: 3-way split-precision accumulation
    f8*m8 + phi8*m8 + f8*d8 where f8/phi8 and m8/d8 are the fp8 value +
    fp8-of-residual of feat and memT — ~3e-3 worst-case score error at
    2.4x less DMA and 25% less PE time than a bf16 matmul pair.
Then, over 1024-col double-bank PSUM tiles:
  - camera sum of exp(score/TEMP - mhat): fused exp+accumulate on the
    scalar engine, in place on PSUM (5 partial sums per row-tile, the
    first 1024-col tile split in two so the act stream starts early;
    host adds them).  mhat is a host-computed per-row bias, identical on
    all cores, statistically pinned to the row max.
  - per-64-proxy-window max of sims' via one DVE windowed tensor_reduce
    per 1024-col tile, direct on PSUM (16 windows per tile; the window
    POSITION identifies the proxies, so no max_index pass is needed)
The host merges the 8 cores' partials into the exact loss: the union of
the top-J windows per row provably covers every proxy the reference's
top-k selections can touch (a window containing the k-th largest value
always ranks within the top-k windows by window-max), and the host
recomputes exact fp32 scores at the candidate proxies so no selection
decision depends on fp8 rounding (fp8 only perturbs WHICH windows are
expanded; margins JG/DELTA absorb its ~0.05 value noise).
"""

import sys

import numpy as np

sys.path.insert(0, "/opt/trn_rl_repo")

# ---- problem constants (hardcoded per spec) ----
P = 32768
D = 256
C = 8
B = 256
TEMP = 0.05
BG_KNN = 50
POSK = 3
BAL_W = 0.15
RATIO = (1.0 - BAL_W) / BAL_W        # 5.666...: sims' = score + RATIO*q  (same order as sims)
INV_TEMP = 1.0 / TEMP                # 20.0
NCORES = 8
PSH = P // NCORES                    # 4096 proxies per core (= one camera)
WIN = 64                             # proxies per candidate window
NWIN_CORE = PSH // WIN               # 64 windows per core
NT = 4                               # 1024-col tiles per row-tile
TW = PSH // NT                       # 1024 cols per tile
# score/activation subtiles: the act stream is saturated, so its TOTAL
# length sets the wall.  Row-tile 0 leads the stream, so its first 1024-col
# tile is split in two (the stream starts as soon as one 512-col half has
# arrived); row-tile 1's subtiles are never start-gating, so it uses four
# 1024-col acts (less per-instruction overhead).  rt1's unused 5th partial-
# sum column is memset to zero.
ACTS_RT = [
    [(0, 512), (512, 1024), (1024, 2048), (2048, 3072), (3072, 4096)],
    [(0, 1024), (1024, 2048), (2048, 3072), (3072, 4096)],
]
NACT = len(ACTS_RT[0])               # 5 partial-sum columns in the output
OBW = NWIN_CORE + NACT               # out cols per row-tile: 64 maxes + 5 partial sums

_CACHE = {}


def _build_bass():
    import concourse.bacc as bacc
    import concourse.mybir as mybir
    import concourse.tile as tile
    from contextlib import ExitStack

    f32 = mybir.dt.float32
    bf16 = mybir.dt.bfloat16
    f8 = mybir.dt.float8e4
    AF = mybir.ActivationFunctionType

    nc = bacc.Bacc("TRN2", target_bir_lowering=False, debug=False)

    # fp8 pack: [f8(feat) | phi8(feat residual) | sims lhsT | m8(memT)] per
    # k-half.  The score matmul runs as a 3-way fp8 split (f8*m8 + phi8*m8 +
    # f8*d8, error ~3e-3 on unit-scale scores) so the whole kernel feeds from
    # fp8 — no bf16 pack at all, and m8 is shared with the sims matmul.
    PK8 = 3 * 256 + PSH                   # 4864
    pack8_d = nc.dram_tensor("pack8", [D, PK8], f8, kind="ExternalInput")
    packd_d = nc.dram_tensor("packd", [D, PSH], f8, kind="ExternalInput")
    nbias_d = nc.dram_tensor("nbias", [B, 1], f32, kind="ExternalInput")
    ob_d = nc.dram_tensor("ob", [B, OBW], f32, kind="ExternalOutput")

    with tile.TileContext(nc) as tc, ExitStack() as ctx:
        consts = ctx.enter_context(tc.tile_pool(name="consts", bufs=1))
        pqp = ctx.enter_context(tc.tile_pool(name="pq", bufs=2, space="PSUM"))
        psp = ctx.enter_context(tc.tile_pool(name="psc", bufs=2, space="PSUM"))
        small = ctx.enter_context(tc.tile_pool(name="small", bufs=2))
        outp = ctx.enter_context(tc.tile_pool(name="outp", bufs=2))

        pack8_sb = consts.tile([128, 2 * PK8], f8, tag="pack8")
        packd_sb = consts.tile([128, 2 * PSH], f8, tag="packd")
        p8_src = pack8_d.rearrange("(k p) c -> p k c", k=2)
        p8_dst = pack8_sb.rearrange("p (k c) -> p k c", k=2)
        pd_src = packd_d.rearrange("(k p) c -> p k c", k=2)
        pd_dst = packd_sb.rearrange("p (k c) -> p k c", k=2)

        # DMA order = first-use order: each tile consumes [m8 tile cols] then
        # [d8 tile cols] (the d8 matmul is last in the score accumulation)
        nc.sync.dma_start(out=p8_dst[:, :, 0:1280], in_=p8_src[:, :, 0:1280])
        nc.sync.dma_start(out=pd_dst[:, :, 0:512], in_=pd_src[:, :, 0:512])
        nc.sync.dma_start(out=p8_dst[:, :, 1280:1792], in_=p8_src[:, :, 1280:1792])
        nc.sync.dma_start(out=pd_dst[:, :, 512:1024], in_=pd_src[:, :, 512:1024])
        for g in range(1, 4):
            lo, hi = 768 + g * 1024, 768 + (g + 1) * 1024
            nc.sync.dma_start(out=p8_dst[:, :, lo:hi], in_=p8_src[:, :, lo:hi])
            nc.sync.dma_start(out=pd_dst[:, :, lo - 768:hi - 768],
                              in_=pd_src[:, :, lo - 768:hi - 768])

        # warm scratch init on the otherwise-idle gpsimd queue, BEFORE the
        # nbias DMAs so the PE warm-up isn't blocked behind them.  The warm
        # matmuls keep the PE busy from ~1us so the clock is ramped when the
        # real matmuls start; a dummy activation preloads the Exp table.
        warm_in = consts.tile([128, 656], bf16, tag="warm")
        nc.gpsimd.memset(warm_in[:], 0.0)
        # Exp-table preload reads a spare slice of the warm tile through a
        # bitcast (disjoint from the matmul operands so the slow table load
        # doesn't chain in front of the warm-up) — no second memset needed
        warm_f = warm_in[:, 640:656].bitcast(f32)
        nc.scalar.activation(warm_f, warm_f, AF.Exp)

        # per-row-tile -mhat bias (tiny; idle gpsimd queue, needed later)
        mbneg = []
        for rt in range(2):
            t = small.tile([128, 1], f32, tag=f"mbneg{rt}", name=f"mbneg_{rt}")
            nc.gpsimd.dma_start(out=t[:], in_=nbias_d[rt * 128:(rt + 1) * 128, :])
            mbneg.append(t)

        wl, wr = warm_in[:, 0:128], warm_in[:, 128:640]
        warm_ps = pqp.tile([128, 512], f32, tag="ps", name="warm_ps")
        for _ in range(4):
            nc.tensor.matmul(warm_ps[:], lhsT=wl, rhs=wr, start=True, stop=True)

        # one SBUF tile holds both row-tiles' outputs so each of the two
        # output DMAs is a single 3D-AP instruction (avoids SP.SEQ/HWDGE
        # serialization of four per-rt DMAs at the tail)
        ob_sb = outp.tile([128, 2 * OBW], f32, tag="ob", name="ob_sb")
        ob_t = [ob_sb[:, 0:OBW], ob_sb[:, OBW:2 * OBW]]
        # rt1 emits only 4 partial sums; zero its unused 5th column
        nc.gpsimd.memset(ob_sb[:, 2 * OBW - 1: 2 * OBW], 0.0)

        pack8_3d = pack8_sb.rearrange("p (k c) -> p k c", k=2)
        packd_3d = packd_sb.rearrange("p (k c) -> p k c", k=2)

        def w8_ap(sec, rt):                    # fp8 lhsT [128, 2, 128]
            base = sec * 256 + rt * 128        # sec: 0=f8, 1=phi8, 2=sims
            return pack8_3d[:, :, base: base + 128]

        def rhs8(lo, hi):                      # fp8 m8 cols [128, 2, w]
            return pack8_3d[:, :, 768 + lo: 768 + hi]

        def rhsd(lo, hi):                      # fp8 d8 cols [128, 2, w]
            return packd_3d[:, :, lo:hi]

        DR = mybir.MatmulPerfMode.DoubleRow

        for t in range(NT):
            lo = t * TW
            for rt in range(2):  # row-tile: batch rows [rt*128, rt*128+128)
                ps_q = pqp.tile([128, TW], f32, tag="ps", name=f"psq_{t}_{rt}")
                # fp8 DoubleRow: both k-halves in one pass, 512 cols per
                # instruction (fp8 moving-operand limit)
                for h in range(2):
                    nc.tensor.matmul(
                        ps_q[:, h * 512:(h + 1) * 512],
                        lhsT=w8_ap(2, rt),
                        rhs=rhs8(lo + h * 512, lo + (h + 1) * 512),
                        start=True, stop=True, perf_mode=DR,
                    )
                # window maxes of sims': one DVE scan over both banks
                nc.vector.tensor_reduce(
                    out=ob_t[rt][:, t * (TW // WIN): (t + 1) * (TW // WIN)],
                    in_=ps_q[:].rearrange("p (w c) -> p w c", c=WIN),
                    axis=mybir.AxisListType.X,
                    op=mybir.AluOpType.max,
                )
                # score via 3-way fp8 split accumulation per 512-col bank
                # half: f8*m8 + phi8*m8 + f8*d8 (the dropped phi8*d8 term is
                # ~1e-4); one fused in-place exp/accumulate per ACT subtile
                # (PSUM out beats the SBUF write port; only the accum_out
                # column is consumed)
                for a, (alo, ahi) in enumerate(ACTS_RT[rt]):
                    if not (alo >= lo and ahi <= lo + TW):
                        continue
                    ps_s = psp.tile([128, ahi - alo], f32, tag="ps",
                                    name=f"pss_{a}_{rt}")
                    for h in range((ahi - alo) // 512):
                        hl, hh = alo + h * 512, alo + (h + 1) * 512
                        parts = [(0, rhs8(hl, hh)), (1, rhs8(hl, hh)),
                                 (0, rhsd(hl, hh))]
                        for i, (sec, rhs) in enumerate(parts):
                            nc.tensor.matmul(
                                ps_s[:, h * 512:(h + 1) * 512],
                                lhsT=w8_ap(sec, rt), rhs=rhs,
                                start=(i == 0), stop=(i == len(parts) - 1),
                                perf_mode=DR,
                            )
                    nc.scalar.activation(
                        ps_s[:], ps_s[:], AF.Exp,
                        bias=mbneg[rt][:], scale=INV_TEMP,
                        accum_out=ob_t[rt][:, NWIN_CORE + a: NWIN_CORE + a + 1],
                    )

        # v8 windows ship as soon as the DVE stream drains; the tiny zpart
        # columns (gated by the later-finishing act stream) go separately so
        # the final transfer is minimal.  Both are single 3D-AP DMAs; the
        # windows ride the Pool SWDGE path so their descriptor prep doesn't
        # hold the HWDGE device in front of the final zpart DMA.
        ob_dst = ob_d.rearrange("(t p) c -> p t c", t=2)
        ob_src = ob_sb.rearrange("p (t c) -> p t c", t=2)
        nc.sync.dma_start(out=ob_dst[:, :, :NWIN_CORE],
                          in_=ob_src[:, :, :NWIN_CORE])
        nc.sync.dma_start(out=ob_dst[:, :, NWIN_CORE:],
                          in_=ob_src[:, :, NWIN_CORE:])

    nc.compile()
    return nc


def _get_nc():
    if "nc" not in _CACHE:
        _CACHE["nc"] = _build_bass()
    return _CACHE["nc"]


def _run_device(in_maps, trace=False):
    from concourse.bass_utils import run_bass_kernel_spmd

    nc = _get_nc()
    res = run_bass_kernel_spmd(
        nc, in_maps, core_ids=list(range(NCORES)), trace=trace
    )
    return res


def kernel(features, targets, cams, epoch, global_memory, all_pseudo_label,
           all_proxy_label, cam_proxies, label_proxies, _want_trace=False):
    import ml_dtypes

    feat = np.ascontiguousarray(np.asarray(features), dtype=np.float32)
    mem = np.ascontiguousarray(np.asarray(global_memory), dtype=np.float32)
    targets = np.asarray(targets).astype(np.int64)
    cams_h = np.asarray(cams).astype(np.int64)
    apl = np.asarray(all_proxy_label).astype(np.int64)
    apsl = np.asarray(all_pseudo_label).astype(np.int64)
    cam_prox = np.asarray(cam_proxies).astype(np.int64)   # [C, PSH]
    lab_prox = np.asarray(label_proxies).astype(np.int64)

    prx = apl[targets]                      # [B] target proxy
    pseudo_y = apsl[targets]                # [B]
    pos_cols = lab_prox[pseudo_y]           # [B, C] positive proxies (cross)
    memprx = mem[prx]                       # [B, D]

    # camera of each proxy; core k owns exactly camera k's proxies
    cam_of_p = np.empty(P, np.int64)
    cam_of_p[cam_prox.reshape(-1)] = np.repeat(np.arange(C), cam_prox.shape[1])
    assert cam_prox.shape == (C, PSH)

    memT = mem.T                            # [D, P]
    lhs2 = feat + np.float32(RATIO) * memprx           # [B, D] sims rows
    # Row-global exp bias: x = INV_TEMP * feat@mem_p with unit mem rows, so
    # x_row ~ N(0, (INV_TEMP*|feat|/sqrt(D))^2).  4.5 sigma sits within
    # [x_max - 80, x_max + ~25] for a 32768-sample max, so exp(x - mhat)
    # neither overflows nor flushes any term that matters.  Identical across
    # cores, so the merge is a plain sum.
    mhat = (4.5 * INV_TEMP / np.sqrt(D)) * np.linalg.norm(
        feat.astype(np.float64), axis=1)    # [B]
    nbias = np.ascontiguousarray((-mhat[:, None]).astype(np.float32))
    f8dt = ml_dtypes.float8_e4m3
    featT = np.ascontiguousarray(feat.T)                # [D, B]
    f8 = featT.astype(f8dt)                             # fp8(feat)
    phi8 = (featT - f8.astype(np.float32)).astype(f8dt)  # fp8 residual
    s8 = lhs2.T.astype(f8dt)                            # sims lhsT
    in_maps = []
    for k in range(NCORES):
        shard = np.ascontiguousarray(memT[:, cam_prox[k]])  # [D, 4096]
        m8 = shard.astype(f8dt)
        d8 = (shard - m8.astype(np.float32)).astype(f8dt)
        pack8 = np.hstack([f8, phi8, s8, m8])           # [D, 4864]
        in_maps.append({
            "pack8": np.ascontiguousarray(pack8),
            "packd": np.ascontiguousarray(d8),
            "nbias": nbias,
        })

    res = _run_device(in_maps, trace=_want_trace)
    results = res.results
    if _want_trace:
        _CACHE["last_exec_time_ns"] = res.exec_time_ns

    ob = np.stack([r["ob"] for r in results]).astype(np.float64)  # [K, B, OBW]
    zpart = ob[:, :, NWIN_CORE:]                                  # [K, B, NT]
    v8 = ob[:, :, :NWIN_CORE]                                     # [K, B, 64]

    rows = np.arange(B)

    # ---- logsumexp merge (cross / intra) ----
    mhat_used = -nbias[:, 0].astype(np.float64)               # exact bias device used
    Zc = zpart.sum(axis=2).T                                  # [B, C] (core k = cam k)
    lse_full = mhat_used + np.log(Zc.sum(axis=1))             # logsumexp over all P of x
    lse_cam = mhat_used + np.log(Zc[rows, cams_h])            # over own camera's proxies

    x_prx = INV_TEMP * np.einsum("bd,bd->b", feat.astype(np.float64),
                                 memprx.astype(np.float64))
    # If a sample's camera does not own its target proxy (possible when cams
    # is generated independently of targets), the reference's one-hot mask is
    # all-zero and its intra term is exactly 0.
    present = cam_of_p[prx] == cams_h
    intra = np.where(present, lse_cam - x_prx, 0.0)

    x_pos = INV_TEMP * np.einsum("bd,bkd->bk", feat.astype(np.float64),
                                 mem[pos_cols].astype(np.float64))
    cross = lse_full - x_pos.mean(axis=1)

    # ---- online loss ----
    # v8[k, b, w] = fp8-accurate max of sims' over window w of core/camera k
    # (proxies cam_prox[k, w*64 .. +64]).  Select candidate windows per row:
    # the global top windows (covers the reference's top-(BG_KNN+POSK)
    # proxies: the window holding the k-th largest value always ranks within
    # the top-k windows) plus every window within DELTA of its camera's best
    # (covers per-camera argmax).  Expand the selected windows and recompute
    # exact fp32 sims'/x there.  Margins sized for fp8 matmul noise
    # (sigma ~0.05 on window maxes).
    W = NCORES * NWIN_CORE                                    # 512 windows/row
    wv = np.moveaxis(v8, 0, 1).reshape(B, W)                  # [B, 512] k-major
    cam_of_w = np.repeat(np.arange(C), NWIN_CORE)             # [512]
    DELTA = 0.4
    JG = 88                                                   # global windows
    cammax = wv.reshape(B, C, NWIN_CORE).max(axis=2)          # [B, C]
    boost = wv >= (cammax[:, cam_of_w] - DELTA)               # near-camera-top
    nboost = int(boost.sum(axis=1).max())
    J = JG + max(nboost, C)
    prio = wv + 1e9 * boost
    sel_w = np.argpartition(-prio, J - 1, axis=1)[:, :J]      # [B, J] unique

    k_of = sel_w // NWIN_CORE                                 # camera/core
    w_of = sel_w % NWIN_CORE
    pid = cam_prox[k_of[:, :, None],
                   (w_of * WIN)[:, :, None] + np.arange(WIN)[None, None, :]]
    pid_b = pid.reshape(B, J * WIN)
    cam_of_cand = np.repeat(cam_of_w[sel_w], WIN, axis=1)     # [B, J*WIN]

    # exact fp32 recompute at the candidate proxies (row-chunked: the
    # gather is the memory hog)
    NCAND = J * WIN
    s_cand = np.empty((B, NCAND), np.float32)
    q_cand = np.empty((B, NCAND), np.float32)
    for lo in range(0, B, 32):
        hi = lo + 32
        memg = mem[pid_b[lo:hi]]                              # [32, NCAND, D]
        s_cand[lo:hi] = np.einsum("bd,bjd->bj", feat[lo:hi], memg)
        q_cand[lo:hi] = np.einsum("bd,bjd->bj", memprx[lo:hi], memg)
    simsp = s_cand.astype(np.float64) + RATIO * q_cand.astype(np.float64)
    x_cand = INV_TEMP * s_cand.astype(np.float64)

    # per-camera global argmax over candidates (exact values)
    tops_val = np.full((B, C), -np.inf)
    tops_j = np.zeros((B, C), np.int64)
    for c in range(C):
        sub = np.where(cam_of_cand == c, simsp, -np.inf)
        a = sub.argmax(axis=1)
        tops_j[:, c] = a
        tops_val[:, c] = sub[rows, a]

    # top-3 cameras by their best sims'
    order = np.argsort(-tops_val, axis=1)[:, :POSK]           # [B, 3]
    chosen_j = np.take_along_axis(tops_j, order, axis=1)      # [B, 3] cand idx
    chosen_pid = np.take_along_axis(pid_b, chosen_j, axis=1)  # [B, 3]

    # top-50 of the remaining candidates (windows are disjoint, so every
    # candidate proxy appears once; only the chosen need masking)
    is_chosen = (pid_b[:, :, None] == chosen_pid[:, None, :]).any(axis=2)
    Vmask = np.where(is_chosen, -np.inf, simsp)
    sel_idx = np.argpartition(-Vmask, BG_KNN, axis=1)[:, :BG_KNN]     # [B, 50]

    x_chosen = np.take_along_axis(x_cand, chosen_j, axis=1)   # [B, 3]
    x_sel = np.take_along_axis(x_cand, sel_idx, axis=1)       # [B, 50]
    xA = np.concatenate([x_chosen, x_sel], axis=1)            # [B, 53]
    mA = xA.max(axis=1)
    lse3 = mA + np.log(np.exp(xA - mA[:, None]).sum(axis=1))
    online = lse3 - x_chosen.mean(axis=1)

    # ---- camera-mean-sum ----
    dbg = globals().get("_DEBUG_COMPS")
    if dbg is not None:
        dbg["intra"] = intra.copy()
        dbg["cross"] = cross.copy()
        dbg["online"] = online.copy()
    total = 0.0
    for c in range(C):
        m = cams_h == c
        if m.any():
            total += intra[m].mean() + cross[m].mean() + online[m].mean()
    return np.float32(total)


# revision 36
# speedup vs baseline: 1.0195x; 1.0089x over previous
"""CameraAwareMemory loss kernel for 8 Trainium2 NeuronCores.

Strategy: camera-sharding — core k owns ALL 4096 proxies of camera k
(P=32768, C=8).  Everything feeds from fp8(e4m3) DoubleRow matmuls (the
256-deep contraction rides two packed k-halves, one instruction per
512-col bank half):
  - sims' = (feat + r*mem[prx]) @ memT: single-fp8 matmul (selection
    noise only; see below)
  - score = feat @ memT: 3-way split-precision accumulation
    f8*m8 + phi8*m8 + f8*d8 where f8/phi8 and m8/d8 are the fp8 value +
    fp8-of-residual of feat and memT — ~3e-3 worst-case score error at
    2.4x less DMA and 25% less PE time than a bf16 matmul pair.
Then, over 1024-col double-bank PSUM tiles:
  - camera sum of exp(score/TEMP - mhat): fused exp+accumulate on the
    scalar engine, in place on PSUM (5 partial sums per row-tile, the
    first 1024-col tile split in two so the act stream starts early;
    host adds them).  mhat is a host-computed per-row bias, identical on
    all cores, statistically pinned to the row max.
  - per-64-proxy-window max of sims' via one DVE windowed tensor_reduce
    per 1024-col tile, direct on PSUM (16 windows per tile; the window
    POSITION identifies the proxies, so no max_index pass is needed)
The host merges the 8 cores' partials into the exact loss: the union of
the top-J windows per row provably covers every proxy the reference's
top-k selections can touch (a window containing the k-th largest value
always ranks within the top-k windows by window-max), and the host
recomputes exact fp32 scores at the candidate proxies so no selection
decision depends on fp8 rounding (fp8 only perturbs WHICH windows are
expanded; margins JG/DELTA absorb its ~0.05 value noise).
"""

import sys

import numpy as np

sys.path.insert(0, "/opt/trn_rl_repo")

# ---- problem constants (hardcoded per spec) ----
P = 32768
D = 256
C = 8
B = 256
TEMP = 0.05
BG_KNN = 50
POSK = 3
BAL_W = 0.15
RATIO = (1.0 - BAL_W) / BAL_W        # 5.666...: sims' = score + RATIO*q  (same order as sims)
INV_TEMP = 1.0 / TEMP                # 20.0
NCORES = 8
PSH = P // NCORES                    # 4096 proxies per core (= one camera)
WIN = 64                             # proxies per candidate window
NWIN_CORE = PSH // WIN               # 64 windows per core
NT = 4                               # 1024-col tiles per row-tile
TW = PSH // NT                       # 1024 cols per tile
# score/activation subtiles: the act stream is saturated, so its TOTAL
# length sets the wall.  Row-tile 0 leads the stream, so its first 1024-col
# tile is split in two (the stream starts as soon as one 512-col half has
# arrived); row-tile 1's subtiles are never start-gating, so it uses four
# 1024-col acts (less per-instruction overhead).  rt1's unused 5th partial-
# sum column is memset to zero.
ACTS_RT = [
    [(0, 512), (512, 1024), (1024, 2048), (2048, 3072), (3072, 4096)],
    [(0, 1024), (1024, 2048), (2048, 3072), (3072, 4096)],
]
NACT = len(ACTS_RT[0])               # 5 partial-sum columns in the output
OBW = NWIN_CORE + NACT               # out cols per row-tile: 64 maxes + 5 partial sums

_CACHE = {}


def _build_bass():
    import concourse.bacc as bacc
    import concourse.mybir as mybir
    import concourse.tile as tile
    from contextlib import ExitStack

    f32 = mybir.dt.float32
    bf16 = mybir.dt.bfloat16
    f8 = mybir.dt.float8e4
    AF = mybir.ActivationFunctionType

    nc = bacc.Bacc("TRN2", target_bir_lowering=False, debug=False)

    # fp8 pack: [f8(feat) | phi8(feat residual) | sims lhsT | m8(memT)] per
    # k-half.  The score matmul runs as a 3-way fp8 split (f8*m8 + phi8*m8 +
    # f8*d8, error ~3e-3 on unit-scale scores) so the whole kernel feeds from
    # fp8 — no bf16 pack at all, and m8 is shared with the sims matmul.
    PK8 = 3 * 256 + PSH                   # 4864
    pack8_d = nc.dram_tensor("pack8", [D, PK8], f8, kind="ExternalInput")
    packd_d = nc.dram_tensor("packd", [D, PSH], f8, kind="ExternalInput")
    nbias_d = nc.dram_tensor("nbias", [B, 1], f32, kind="ExternalInput")
    ob_d = nc.dram_tensor("ob", [B, OBW], f32, kind="ExternalOutput")

    with tile.TileContext(nc) as tc, ExitStack() as ctx:
        consts = ctx.enter_context(tc.tile_pool(name="consts", bufs=1))
        pqp = ctx.enter_context(tc.tile_pool(name="pq", bufs=2, space="PSUM"))
        psp = ctx.enter_context(tc.tile_pool(name="psc", bufs=2, space="PSUM"))
        small = ctx.enter_context(tc.tile_pool(name="small", bufs=2))
        outp = ctx.enter_context(tc.tile_pool(name="outp", bufs=2))

        pack8_sb = consts.tile([128, 2 * PK8], f8, tag="pack8")
        packd_sb = consts.tile([128, 2 * PSH], f8, tag="packd")
        p8_src = pack8_d.rearrange("(k p) c -> p k c", k=2)
        p8_dst = pack8_sb.rearrange("p (k c) -> p k c", k=2)
        pd_src = packd_d.rearrange("(k p) c -> p k c", k=2)
        pd_dst = packd_sb.rearrange("p (k c) -> p k c", k=2)

        # DMA order = first-use order: each tile consumes [m8 tile cols] then
        # [d8 tile cols] (the d8 matmul is last in the score accumulation)
        nc.sync.dma_start(out=p8_dst[:, :, 0:1280], in_=p8_src[:, :, 0:1280])
        nc.sync.dma_start(out=pd_dst[:, :, 0:512], in_=pd_src[:, :, 0:512])
        nc.sync.dma_start(out=p8_dst[:, :, 1280:1792], in_=p8_src[:, :, 1280:1792])
        nc.sync.dma_start(out=pd_dst[:, :, 512:1024], in_=pd_src[:, :, 512:1024])
        for g in range(1, 4):
            lo, hi = 768 + g * 1024, 768 + (g + 1) * 1024
            nc.sync.dma_start(out=p8_dst[:, :, lo:hi], in_=p8_src[:, :, lo:hi])
            nc.sync.dma_start(out=pd_dst[:, :, lo - 768:hi - 768],
                              in_=pd_src[:, :, lo - 768:hi - 768])

        # warm scratch init on the otherwise-idle gpsimd queue, BEFORE the
        # nbias DMAs so the PE warm-up isn't blocked behind them.  The warm
        # matmuls keep the PE busy from ~1us so the clock is ramped when the
        # real matmuls start; a dummy activation preloads the Exp table.
        warm_in = consts.tile([128, 656], bf16, tag="warm")
        nc.gpsimd.memset(warm_in[:], 0.0)
        # Exp-table preload reads a spare slice of the warm tile through a
        # bitcast (disjoint from the matmul operands so the slow table load
        # doesn't chain in front of the warm-up) — no second memset needed
        warm_f = warm_in[:, 640:656].bitcast(f32)
        nc.scalar.activation(warm_f, warm_f, AF.Exp)

        # per-row-tile -mhat bias (tiny; idle gpsimd queue, needed later)
        mbneg = []
        for rt in range(2):
            t = small.tile([128, 1], f32, tag=f"mbneg{rt}", name=f"mbneg_{rt}")
            nc.gpsimd.dma_start(out=t[:], in_=nbias_d[rt * 128:(rt + 1) * 128, :])
            mbneg.append(t)

        wl, wr = warm_in[:, 0:128], warm_in[:, 128:640]
        warm_ps = pqp.tile([128, 512], f32, tag="ps", name="warm_ps")
        for _ in range(4):
            nc.tensor.matmul(warm_ps[:], lhsT=wl, rhs=wr, start=True, stop=True)

        # one SBUF tile holds both row-tiles' outputs so each of the two
        # output DMAs is a single 3D-AP instruction (avoids SP.SEQ/HWDGE
        # serialization of four per-rt DMAs at the tail)
        ob_sb = outp.tile([128, 2 * OBW], f32, tag="ob", name="ob_sb")
        ob_t = [ob_sb[:, 0:OBW], ob_sb[:, OBW:2 * OBW]]
        # rt1 emits only 4 partial sums; zero its unused 5th column
        nc.gpsimd.memset(ob_sb[:, 2 * OBW - 1: 2 * OBW], 0.0)

        pack8_3d = pack8_sb.rearrange("p (k c) -> p k c", k=2)
        packd_3d = packd_sb.rearrange("p (k c) -> p k c", k=2)

        def w8_ap(sec, rt):                    # fp8 lhsT [128, 2, 128]
            base = sec * 256 + rt * 128        # sec: 0=f8, 1=phi8, 2=sims
            return pack8_3d[:, :, base: base + 128]

        def rhs8(lo, hi):                      # fp8 m8 cols [128, 2, w]
            return pack8_3d[:, :, 768 + lo: 768 + hi]

        def rhsd(lo, hi):                      # fp8 d8 cols [128, 2, w]
            return packd_3d[:, :, lo:hi]

        DR = mybir.MatmulPerfMode.DoubleRow

        for t in range(NT):
            lo = t * TW
            for rt in range(2):  # row-tile: batch rows [rt*128, rt*128+128)
                ps_q = pqp.tile([128, TW], f32, tag="ps", name=f"psq_{t}_{rt}")
                # fp8 DoubleRow: both k-halves in one pass, 512 cols per
                # instruction (fp8 moving-operand limit)
                for h in range(2):
                    nc.tensor.matmul(
                        ps_q[:, h * 512:(h + 1) * 512],
                        lhsT=w8_ap(2, rt),
                        rhs=rhs8(lo + h * 512, lo + (h + 1) * 512),
                        start=True, stop=True, perf_mode=DR,
                    )
                # window maxes of sims': one DVE scan over both banks
                nc.vector.tensor_reduce(
                    out=ob_t[rt][:, t * (TW // WIN): (t + 1) * (TW // WIN)],
                    in_=ps_q[:].rearrange("p (w c) -> p w c", c=WIN),
                    axis=mybir.AxisListType.X,
                    op=mybir.AluOpType.max,
                )
                # score via 3-way fp8 split accumulation per 512-col bank
                # half: f8*m8 + phi8*m8 + f8*d8 (the dropped phi8*d8 term is
                # ~1e-4); one fused in-place exp/accumulate per ACT subtile
                # (PSUM out beats the SBUF write port; only the accum_out
                # column is consumed)
                for a, (alo, ahi) in enumerate(ACTS_RT[rt]):
                    if not (alo >= lo and ahi <= lo + TW):
                        continue
                    ps_s = psp.tile([128, ahi - alo], f32, tag="ps",
                                    name=f"pss_{a}_{rt}")
                    for h in range((ahi - alo) // 512):
                        hl, hh = alo + h * 512, alo + (h + 1) * 512
                        parts = [(0, rhs8(hl, hh)), (1, rhs8(hl, hh)),
                                 (0, rhsd(hl, hh))]
                        for i, (sec, rhs) in enumerate(parts):
                            nc.tensor.matmul(
                                ps_s[:, h * 512:(h + 1) * 512],
                                lhsT=w8_ap(sec, rt), rhs=rhs,
                                start=(i == 0), stop=(i == len(parts) - 1),
                                perf_mode=DR,
                            )
                    nc.scalar.activation(
                        ps_s[:], ps_s[:], AF.Exp,
                        bias=mbneg[rt][:], scale=INV_TEMP,
                        accum_out=ob_t[rt][:, NWIN_CORE + a: NWIN_CORE + a + 1],
                    )

        # Output (all single 3D-AP DMAs): the bulk of the windows (t0-t2)
        # ships early, clearing the HWDGE device well before the tail; the
        # last tile's windows ride the Pool SWDGE path so their descriptor
        # prep runs in PARALLEL with the final zpart DMA's HWDGE prep (both
        # fire within ~350ns of each other at the very end).
        W3 = 3 * (TW // WIN)                    # windows of tiles t0-t2
        ob_dst = ob_d.rearrange("(t p) c -> p t c", t=2)
        ob_src = ob_sb.rearrange("p (t c) -> p t c", t=2)
        nc.sync.dma_start(out=ob_dst[:, :, :W3], in_=ob_src[:, :, :W3])
        nc.gpsimd.dma_start(out=ob_dst[:, :, W3:NWIN_CORE],
                            in_=ob_src[:, :, W3:NWIN_CORE])
        nc.sync.dma_start(out=ob_dst[:, :, NWIN_CORE:],
                          in_=ob_src[:, :, NWIN_CORE:])

    nc.compile()
    return nc


def _get_nc():
    if "nc" not in _CACHE:
        _CACHE["nc"] = _build_bass()
    return _CACHE["nc"]


def _run_device(in_maps, trace=False):
    from concourse.bass_utils import run_bass_kernel_spmd

    nc = _get_nc()
    res = run_bass_kernel_spmd(
        nc, in_maps, core_ids=list(range(NCORES)), trace=trace
    )
    return res


def kernel(features, targets, cams, epoch, global_memory, all_pseudo_label,
           all_proxy_label, cam_proxies, label_proxies, _want_trace=False):
    import ml_dtypes

    feat = np.ascontiguousarray(np.asarray(features), dtype=np.float32)
    mem = np.ascontiguousarray(np.asarray(global_memory), dtype=np.float32)
    targets = np.asarray(targets).astype(np.int64)
    cams_h = np.asarray(cams).astype(np.int64)
    apl = np.asarray(all_proxy_label).astype(np.int64)
    apsl = np.asarray(all_pseudo_label).astype(np.int64)
    cam_prox = np.asarray(cam_proxies).astype(np.int64)   # [C, PSH]
    lab_prox = np.asarray(label_proxies).astype(np.int64)

    prx = apl[targets]                      # [B] target proxy
    pseudo_y = apsl[targets]                # [B]
    pos_cols = lab_prox[pseudo_y]           # [B, C] positive proxies (cross)
    memprx = mem[prx]                       # [B, D]

    # camera of each proxy; core k owns exactly camera k's proxies
    cam_of_p = np.empty(P, np.int64)
    cam_of_p[cam_prox.reshape(-1)] = np.repeat(np.arange(C), cam_prox.shape[1])
    assert cam_prox.shape == (C, PSH)

    memT = mem.T                            # [D, P]
    lhs2 = feat + np.float32(RATIO) * memprx           # [B, D] sims rows
    # Row-global exp bias: x = INV_TEMP * feat@mem_p with unit mem rows, so
    # x_row ~ N(0, (INV_TEMP*|feat|/sqrt(D))^2).  4.5 sigma sits within
    # [x_max - 80, x_max + ~25] for a 32768-sample max, so exp(x - mhat)
    # neither overflows nor flushes any term that matters.  Identical across
    # cores, so the merge is a plain sum.
    mhat = (4.5 * INV_TEMP / np.sqrt(D)) * np.linalg.norm(
        feat.astype(np.float64), axis=1)    # [B]
    nbias = np.ascontiguousarray((-mhat[:, None]).astype(np.float32))
    f8dt = ml_dtypes.float8_e4m3
    featT = np.ascontiguousarray(feat.T)                # [D, B]
    f8 = featT.astype(f8dt)                             # fp8(feat)
    phi8 = (featT - f8.astype(np.float32)).astype(f8dt)  # fp8 residual
    s8 = lhs2.T.astype(f8dt)                            # sims lhsT
    in_maps = []
    for k in range(NCORES):
        shard = np.ascontiguousarray(memT[:, cam_prox[k]])  # [D, 4096]
        m8 = shard.astype(f8dt)
        d8 = (shard - m8.astype(np.float32)).astype(f8dt)
        pack8 = np.hstack([f8, phi8, s8, m8])           # [D, 4864]
        in_maps.append({
            "pack8": np.ascontiguousarray(pack8),
            "packd": np.ascontiguousarray(d8),
            "nbias": nbias,
        })

    res = _run_device(in_maps, trace=_want_trace)
    results = res.results
    if _want_trace:
        _CACHE["last_exec_time_ns"] = res.exec_time_ns

    ob = np.stack([r["ob"] for r in results]).astype(np.float64)  # [K, B, OBW]
    zpart = ob[:, :, NWIN_CORE:]                                  # [K, B, NT]
    v8 = ob[:, :, :NWIN_CORE]                                     # [K, B, 64]

    rows = np.arange(B)

    # ---- logsumexp merge (cross / intra) ----
    mhat_used = -nbias[:, 0].astype(np.float64)               # exact bias device used
    Zc = zpart.sum(axis=2).T                                  # [B, C] (core k = cam k)
    lse_full = mhat_used + np.log(Zc.sum(axis=1))             # logsumexp over all P of x
    lse_cam = mhat_used + np.log(Zc[rows, cams_h])            # over own camera's proxies

    x_prx = INV_TEMP * np.einsum("bd,bd->b", feat.astype(np.float64),
                                 memprx.astype(np.float64))
    # If a sample's camera does not own its target proxy (possible when cams
    # is generated independently of targets), the reference's one-hot mask is
    # all-zero and its intra term is exactly 0.
    present = cam_of_p[prx] == cams_h
    intra = np.where(present, lse_cam - x_prx, 0.0)

    x_pos = INV_TEMP * np.einsum("bd,bkd->bk", feat.astype(np.float64),
                                 mem[pos_cols].astype(np.float64))
    cross = lse_full - x_pos.mean(axis=1)

    # ---- online loss ----
    # v8[k, b, w] = fp8-accurate max of sims' over window w of core/camera k
    # (proxies cam_prox[k, w*64 .. +64]).  Select candidate windows per row:
    # the global top windows (covers the reference's top-(BG_KNN+POSK)
    # proxies: the window holding the k-th largest value always ranks within
    # the top-k windows) plus every window within DELTA of its camera's best
    # (covers per-camera argmax).  Expand the selected windows and recompute
    # exact fp32 sims'/x there.  Margins sized for fp8 matmul noise
    # (sigma ~0.05 on window maxes).
    W = NCORES * NWIN_CORE                                    # 512 windows/row
    wv = np.moveaxis(v8, 0, 1).reshape(B, W)                  # [B, 512] k-major
    cam_of_w = np.repeat(np.arange(C), NWIN_CORE)             # [512]
    DELTA = 0.4
    JG = 88                                                   # global windows
    cammax = wv.reshape(B, C, NWIN_CORE).max(axis=2)          # [B, C]
    boost = wv >= (cammax[:, cam_of_w] - DELTA)               # near-camera-top
    nboost = int(boost.sum(axis=1).max())
    J = JG + max(nboost, C)
    prio = wv + 1e9 * boost
    sel_w = np.argpartition(-prio, J - 1, axis=1)[:, :J]      # [B, J] unique

    k_of = sel_w // NWIN_CORE                                 # camera/core
    w_of = sel_w % NWIN_CORE
    pid = cam_prox[k_of[:, :, None],
                   (w_of * WIN)[:, :, None] + np.arange(WIN)[None, None, :]]
    pid_b = pid.reshape(B, J * WIN)
    cam_of_cand = np.repeat(cam_of_w[sel_w], WIN, axis=1)     # [B, J*WIN]

    # exact fp32 recompute at the candidate proxies (row-chunked: the
    # gather is the memory hog)
    NCAND = J * WIN
    s_cand = np.empty((B, NCAND), np.float32)
    q_cand = np.empty((B, NCAND), np.float32)
    for lo in range(0, B, 32):
        hi = lo + 32
        memg = mem[pid_b[lo:hi]]                              # [32, NCAND, D]
        s_cand[lo:hi] = np.einsum("bd,bjd->bj", feat[lo:hi], memg)
        q_cand[lo:hi] = np.einsum("bd,bjd->bj", memprx[lo:hi], memg)
    simsp = s_cand.astype(np.float64) + RATIO * q_cand.astype(np.float64)
    x_cand = INV_TEMP * s_cand.astype(np.float64)

    # per-camera global argmax over candidates (exact values)
    tops_val = np.full((B, C), -np.inf)
    tops_j = np.zeros((B, C), np.int64)
    for c in range(C):
        sub = np.where(cam_of_cand == c, simsp, -np.inf)
        a = sub.argmax(axis=1)
        tops_j[:, c] = a
        tops_val[:, c] = sub[rows, a]

    # top-3 cameras by their best sims'
    order = np.argsort(-tops_val, axis=1)[:, :POSK]           # [B, 3]
    chosen_j = np.take_along_axis(tops_j, order, axis=1)      # [B, 3] cand idx
    chosen_pid = np.take_along_axis(pid_b, chosen_j, axis=1)  # [B, 3]

    # top-50 of the remaining candidates (windows are disjoint, so every
    # candidate proxy appears once; only the chosen need masking)
    is_chosen = (pid_b[:, :, None] == chosen_pid[:, None, :]).any(axis=2)
    Vmask = np.where(is_chosen, -np.inf, simsp)
    sel_idx = np.argpartition(-Vmask, BG_KNN, axis=1)[:, :BG_KNN]     # [B, 50]

    x_chosen = np.take_along_axis(x_cand, chosen_j, axis=1)   # [B, 3]
    x_sel = np.take_along_axis(x_cand, sel_idx, axis=1)       # [B, 50]
    xA = np.concatenate([x_chosen, x_sel], axis=1)            # [B, 53]
    mA = xA.max(axis=1)
    lse3 = mA + np.log(np.exp(xA - mA[:, None]).sum(axis=1))
    online = lse3 - x_chosen.mean(axis=1)

    # ---- camera-mean-sum ----
    dbg = globals().get("_DEBUG_COMPS")
    if dbg is not None:
        dbg["intra"] = intra.copy()
        dbg["cross"] = cross.copy()
        dbg["online"] = online.copy()
    total = 0.0
    for c in range(C):
        m = cams_h == c
        if m.any():
            total += intra[m].mean() + cross[m].mean() + online[m].mean()
    return np.float32(total)


# revision 37
# speedup vs baseline: 1.0360x; 1.0161x over previous
"""CameraAwareMemory loss kernel for 8 Trainium2 NeuronCores.

Strategy: camera-sharding — core k owns ALL 4096 proxies of camera k
(P=32768, C=8).  Everything feeds from fp8(e4m3) DoubleRow matmuls (the
256-deep contraction rides two packed k-halves, one instruction per
512-col bank half):
  - sims' = (feat + r*mem[prx]) @ memT: single-fp8 matmul (selection
    noise only; see below)
  - score = feat @ memT: 3-way split-precision accumulation
    f8*m8 + phi8*m8 + f8*d8 where f8/phi8 and m8/d8 are the fp8 value +
    fp8-of-residual of feat and memT — ~3e-3 worst-case score error at
    2.4x less DMA and 25% less PE time than a bf16 matmul pair.
Then, over 1024-col double-bank PSUM tiles:
  - camera sum of exp(score/TEMP - mhat): fused exp+accumulate on the
    scalar engine, in place on PSUM (5 partial sums per row-tile, the
    first 1024-col tile split in two so the act stream starts early;
    host adds them).  mhat is a host-computed per-row bias, identical on
    all cores, statistically pinned to the row max.
  - per-64-proxy-window max of sims' via one DVE windowed tensor_reduce
    per 1024-col tile, direct on PSUM (16 windows per tile; the window
    POSITION identifies the proxies, so no max_index pass is needed)
The host merges the 8 cores' partials into the exact loss: the union of
the top-J windows per row provably covers every proxy the reference's
top-k selections can touch (a window containing the k-th largest value
always ranks within the top-k windows by window-max), and the host
recomputes exact fp32 scores at the candidate proxies so no selection
decision depends on fp8 rounding (fp8 only perturbs WHICH windows are
expanded; margins JG/DELTA absorb its ~0.05 value noise).
"""

import sys

import numpy as np

sys.path.insert(0, "/opt/trn_rl_repo")

# ---- problem constants (hardcoded per spec) ----
P = 32768
D = 256
C = 8
B = 256
TEMP = 0.05
BG_KNN = 50
POSK = 3
BAL_W = 0.15
RATIO = (1.0 - BAL_W) / BAL_W        # 5.666...: sims' = score + RATIO*q  (same order as sims)
INV_TEMP = 1.0 / TEMP                # 20.0
NCORES = 8
PSH = P // NCORES                    # 4096 proxies per core (= one camera)
WIN = 64                             # proxies per candidate window
NWIN_CORE = PSH // WIN               # 64 windows per core
NT = 4                               # 1024-col tiles per row-tile
TW = PSH // NT                       # 1024 cols per tile
# score/activation subtiles: the act stream is saturated, so its TOTAL
# length sets the wall.  Row-tile 0 leads the stream, so its first 1024-col
# tile is split in two (the stream starts as soon as one 512-col half has
# arrived); row-tile 1's subtiles are never start-gating, so it uses four
# 1024-col acts (less per-instruction overhead).  rt1's unused 5th partial-
# sum column is memset to zero.
ACTS_RT = [
    [(0, 512), (512, 1024), (1024, 2048), (2048, 3072), (3072, 4096)],
    [(0, 1024), (1024, 2048), (2048, 3072), (3072, 4096)],
]
NACT = len(ACTS_RT[0])               # 5 partial-sum columns in the output
OBW = NWIN_CORE + NACT               # out cols per row-tile: 64 maxes + 5 partial sums

_CACHE = {}


def _build_bass():
    import concourse.bacc as bacc
    import concourse.mybir as mybir
    import concourse.tile as tile
    from contextlib import ExitStack

    f32 = mybir.dt.float32
    bf16 = mybir.dt.bfloat16
    f8 = mybir.dt.float8e4
    AF = mybir.ActivationFunctionType

    nc = bacc.Bacc("TRN2", target_bir_lowering=False, debug=False)

    # fp8 pack: [f8(feat) | phi8(feat residual) | sims lhsT | m8(memT)] per
    # k-half.  The score matmul runs as a 3-way fp8 split (f8*m8 + phi8*m8 +
    # f8*d8, error ~3e-3 on unit-scale scores) so the whole kernel feeds from
    # fp8 — no bf16 pack at all, and m8 is shared with the sims matmul.
    PK8 = 3 * 256 + PSH                   # 4864
    pack8_d = nc.dram_tensor("pack8", [D, PK8], f8, kind="ExternalInput")
    packd_d = nc.dram_tensor("packd", [D, PSH], f8, kind="ExternalInput")
    nbias_d = nc.dram_tensor("nbias", [B, 1], f32, kind="ExternalInput")
    ob_d = nc.dram_tensor("ob", [B, OBW], f32, kind="ExternalOutput")

    with tile.TileContext(nc) as tc, ExitStack() as ctx:
        consts = ctx.enter_context(tc.tile_pool(name="consts", bufs=1))
        pqp = ctx.enter_context(tc.tile_pool(name="pq", bufs=2, space="PSUM"))
        psp = ctx.enter_context(tc.tile_pool(name="psc", bufs=2, space="PSUM"))
        small = ctx.enter_context(tc.tile_pool(name="small", bufs=2))
        outp = ctx.enter_context(tc.tile_pool(name="outp", bufs=2))

        pack8_sb = consts.tile([128, 2 * PK8], f8, tag="pack8")
        packd_sb = consts.tile([128, 2 * PSH], f8, tag="packd")
        p8_src = pack8_d.rearrange("(k p) c -> p k c", k=2)
        p8_dst = pack8_sb.rearrange("p (k c) -> p k c", k=2)
        pd_src = packd_d.rearrange("(k p) c -> p k c", k=2)
        pd_dst = packd_sb.rearrange("p (k c) -> p k c", k=2)

        # DMA order = first-use order: each tile consumes [m8 tile cols] then
        # [d8 tile cols] (the d8 matmul is last in the score accumulation)
        nc.sync.dma_start(out=p8_dst[:, :, 0:1280], in_=p8_src[:, :, 0:1280])
        nc.sync.dma_start(out=pd_dst[:, :, 0:512], in_=pd_src[:, :, 0:512])
        nc.sync.dma_start(out=p8_dst[:, :, 1280:1792], in_=p8_src[:, :, 1280:1792])
        nc.sync.dma_start(out=pd_dst[:, :, 512:1024], in_=pd_src[:, :, 512:1024])
        for g in range(1, 4):
            lo, hi = 768 + g * 1024, 768 + (g + 1) * 1024
            nc.sync.dma_start(out=p8_dst[:, :, lo:hi], in_=p8_src[:, :, lo:hi])
            nc.sync.dma_start(out=pd_dst[:, :, lo - 768:hi - 768],
                              in_=pd_src[:, :, lo - 768:hi - 768])

        # warm scratch init on the otherwise-idle gpsimd queue, BEFORE the
        # nbias DMAs so the PE warm-up isn't blocked behind them.  The warm
        # matmuls keep the PE busy from ~1us so the clock is ramped when the
        # real matmuls start; a dummy activation preloads the Exp table.
        warm_in = consts.tile([128, 656], bf16, tag="warm")
        nc.gpsimd.memset(warm_in[:], 0.0)
        # Exp-table preload reads a spare slice of the warm tile through a
        # bitcast (disjoint from the matmul operands so the slow table load
        # doesn't chain in front of the warm-up) — no second memset needed
        warm_f = warm_in[:, 640:656].bitcast(f32)
        nc.scalar.activation(warm_f, warm_f, AF.Exp)

        # per-row-tile -mhat bias (tiny; idle gpsimd queue, needed later)
        mbneg = []
        for rt in range(2):
            t = small.tile([128, 1], f32, tag=f"mbneg{rt}", name=f"mbneg_{rt}")
            nc.gpsimd.dma_start(out=t[:], in_=nbias_d[rt * 128:(rt + 1) * 128, :])
            mbneg.append(t)

        wl, wr = warm_in[:, 0:128], warm_in[:, 128:640]
        warm_ps = pqp.tile([128, 512], f32, tag="ps", name="warm_ps")
        for _ in range(4):
            nc.tensor.matmul(warm_ps[:], lhsT=wl, rhs=wr, start=True, stop=True)

        # one SBUF tile holds both row-tiles' outputs so each of the two
        # output DMAs is a single 3D-AP instruction (avoids SP.SEQ/HWDGE
        # serialization of four per-rt DMAs at the tail)
        ob_sb = outp.tile([128, 2 * OBW], f32, tag="ob", name="ob_sb")
        ob_t = [ob_sb[:, 0:OBW], ob_sb[:, OBW:2 * OBW]]
        # rt1 emits only 4 partial sums; zero its unused 5th column
        nc.gpsimd.memset(ob_sb[:, 2 * OBW - 1: 2 * OBW], 0.0)

        pack8_3d = pack8_sb.rearrange("p (k c) -> p k c", k=2)
        packd_3d = packd_sb.rearrange("p (k c) -> p k c", k=2)

        def w8_ap(sec, rt):                    # fp8 lhsT [128, 2, 128]
            base = sec * 256 + rt * 128        # sec: 0=f8, 1=phi8, 2=sims
            return pack8_3d[:, :, base: base + 128]

        def rhs8(lo, hi):                      # fp8 m8 cols [128, 2, w]
            return pack8_3d[:, :, 768 + lo: 768 + hi]

        def rhsd(lo, hi):                      # fp8 d8 cols [128, 2, w]
            return packd_3d[:, :, lo:hi]

        DR = mybir.MatmulPerfMode.DoubleRow

        for t in range(NT):
            lo = t * TW
            for rt in range(2):  # row-tile: batch rows [rt*128, rt*128+128)
                # fp8 DoubleRow sims matmuls (both k-halves in one pass, 512
                # cols per instruction) + windowed DVE max straight off PSUM.
                # The stream-leading (t0, rt0) pair uses two SEPARATE 512-col
                # tiles so the first reduce depends only on the first half's
                # data — the saturated DVE stream starts (and so drains)
                # ~450ns earlier.
                split = 2 if (t == 0 and rt == 0) else 1
                sw = TW // split
                for s in range(split):
                    ps_q = pqp.tile([128, sw], f32, tag="ps",
                                    name=f"psq_{t}_{rt}_{s}")
                    for h in range(sw // 512):
                        c0 = lo + s * sw + h * 512
                        nc.tensor.matmul(
                            ps_q[:, h * 512:(h + 1) * 512],
                            lhsT=w8_ap(2, rt), rhs=rhs8(c0, c0 + 512),
                            start=True, stop=True, perf_mode=DR,
                        )
                    wbase = (lo + s * sw) // WIN
                    nc.vector.tensor_reduce(
                        out=ob_t[rt][:, wbase: wbase + sw // WIN],
                        in_=ps_q[:].rearrange("p (w c) -> p w c", c=WIN),
                        axis=mybir.AxisListType.X,
                        op=mybir.AluOpType.max,
                    )
                # score via 3-way fp8 split accumulation per 512-col bank
                # half: f8*m8 + phi8*m8 + f8*d8 (the dropped phi8*d8 term is
                # ~1e-4); one fused in-place exp/accumulate per ACT subtile
                # (PSUM out beats the SBUF write port; only the accum_out
                # column is consumed)
                for a, (alo, ahi) in enumerate(ACTS_RT[rt]):
                    if not (alo >= lo and ahi <= lo + TW):
                        continue
                    ps_s = psp.tile([128, ahi - alo], f32, tag="ps",
                                    name=f"pss_{a}_{rt}")
                    for h in range((ahi - alo) // 512):
                        hl, hh = alo + h * 512, alo + (h + 1) * 512
                        parts = [(0, rhs8(hl, hh)), (1, rhs8(hl, hh)),
                                 (0, rhsd(hl, hh))]
                        for i, (sec, rhs) in enumerate(parts):
                            nc.tensor.matmul(
                                ps_s[:, h * 512:(h + 1) * 512],
                                lhsT=w8_ap(sec, rt), rhs=rhs,
                                start=(i == 0), stop=(i == len(parts) - 1),
                                perf_mode=DR,
                            )
                    nc.scalar.activation(
                        ps_s[:], ps_s[:], AF.Exp,
                        bias=mbneg[rt][:], scale=INV_TEMP,
                        accum_out=ob_t[rt][:, NWIN_CORE + a: NWIN_CORE + a + 1],
                    )

        # Output (all single 3D-AP DMAs): the bulk of the windows (t0-t2)
        # ships early, clearing the HWDGE device well before the tail; the
        # last tile's windows ride the Pool SWDGE path so their descriptor
        # prep runs in PARALLEL with the final zpart DMA's HWDGE prep (both
        # fire within ~350ns of each other at the very end).
        W3 = 3 * (TW // WIN)                    # windows of tiles t0-t2
        ob_dst = ob_d.rearrange("(t p) c -> p t c", t=2)
        ob_src = ob_sb.rearrange("p (t c) -> p t c", t=2)
        nc.sync.dma_start(out=ob_dst[:, :, :W3], in_=ob_src[:, :, :W3])
        nc.gpsimd.dma_start(out=ob_dst[:, :, W3:NWIN_CORE],
                            in_=ob_src[:, :, W3:NWIN_CORE])
        nc.sync.dma_start(out=ob_dst[:, :, NWIN_CORE:],
                          in_=ob_src[:, :, NWIN_CORE:])

    nc.compile()
    return nc


def _get_nc():
    if "nc" not in _CACHE:
        _CACHE["nc"] = _build_bass()
    return _CACHE["nc"]


def _run_device(in_maps, trace=False):
    from concourse.bass_utils import run_bass_kernel_spmd

    nc = _get_nc()
    res = run_bass_kernel_spmd(
        nc, in_maps, core_ids=list(range(NCORES)), trace=trace
    )
    return res


def kernel(features, targets, cams, epoch, global_memory, all_pseudo_label,
           all_proxy_label, cam_proxies, label_proxies, _want_trace=False):
    import ml_dtypes

    feat = np.ascontiguousarray(np.asarray(features), dtype=np.float32)
    mem = np.ascontiguousarray(np.asarray(global_memory), dtype=np.float32)
    targets = np.asarray(targets).astype(np.int64)
    cams_h = np.asarray(cams).astype(np.int64)
    apl = np.asarray(all_proxy_label).astype(np.int64)
    apsl = np.asarray(all_pseudo_label).astype(np.int64)
    cam_prox = np.asarray(cam_proxies).astype(np.int64)   # [C, PSH]
    lab_prox = np.asarray(label_proxies).astype(np.int64)

    prx = apl[targets]                      # [B] target proxy
    pseudo_y = apsl[targets]                # [B]
    pos_cols = lab_prox[pseudo_y]           # [B, C] positive proxies (cross)
    memprx = mem[prx]                       # [B, D]

    # camera of each proxy; core k owns exactly camera k's proxies
    cam_of_p = np.empty(P, np.int64)
    cam_of_p[cam_prox.reshape(-1)] = np.repeat(np.arange(C), cam_prox.shape[1])
    assert cam_prox.shape == (C, PSH)

    memT = mem.T                            # [D, P]
    lhs2 = feat + np.float32(RATIO) * memprx           # [B, D] sims rows
    # Row-global exp bias: x = INV_TEMP * feat@mem_p with unit mem rows, so
    # x_row ~ N(0, (INV_TEMP*|feat|/sqrt(D))^2).  4.5 sigma sits within
    # [x_max - 80, x_max + ~25] for a 32768-sample max, so exp(x - mhat)
    # neither overflows nor flushes any term that matters.  Identical across
    # cores, so the merge is a plain sum.
    mhat = (4.5 * INV_TEMP / np.sqrt(D)) * np.linalg.norm(
        feat.astype(np.float64), axis=1)    # [B]
    nbias = np.ascontiguousarray((-mhat[:, None]).astype(np.float32))
    f8dt = ml_dtypes.float8_e4m3
    featT = np.ascontiguousarray(feat.T)                # [D, B]
    f8 = featT.astype(f8dt)                             # fp8(feat)
    phi8 = (featT - f8.astype(np.float32)).astype(f8dt)  # fp8 residual
    s8 = lhs2.T.astype(f8dt)                            # sims lhsT
    in_maps = []
    for k in range(NCORES):
        shard = np.ascontiguousarray(memT[:, cam_prox[k]])  # [D, 4096]
        m8 = shard.astype(f8dt)
        d8 = (shard - m8.astype(np.float32)).astype(f8dt)
        pack8 = np.hstack([f8, phi8, s8, m8])           # [D, 4864]
        in_maps.append({
            "pack8": np.ascontiguousarray(pack8),
            "packd": np.ascontiguousarray(d8),
            "nbias": nbias,
        })

    res = _run_device(in_maps, trace=_want_trace)
    results = res.results
    if _want_trace:
        _CACHE["last_exec_time_ns"] = res.exec_time_ns

    ob = np.stack([r["ob"] for r in results]).astype(np.float64)  # [K, B, OBW]
    zpart = ob[:, :, NWIN_CORE:]                                  # [K, B, NT]
    v8 = ob[:, :, :NWIN_CORE]                                     # [K, B, 64]

    rows = np.arange(B)

    # ---- logsumexp merge (cross / intra) ----
    mhat_used = -nbias[:, 0].astype(np.float64)               # exact bias device used
    Zc = zpart.sum(axis=2).T                                  # [B, C] (core k = cam k)
    lse_full = mhat_used + np.log(Zc.sum(axis=1))             # logsumexp over all P of x
    lse_cam = mhat_used + np.log(Zc[rows, cams_h])            # over own camera's proxies

    x_prx = INV_TEMP * np.einsum("bd,bd->b", feat.astype(np.float64),
                                 memprx.astype(np.float64))
    # If a sample's camera does not own its target proxy (possible when cams
    # is generated independently of targets), the reference's one-hot mask is
    # all-zero and its intra term is exactly 0.
    present = cam_of_p[prx] == cams_h
    intra = np.where(present, lse_cam - x_prx, 0.0)

    x_pos = INV_TEMP * np.einsum("bd,bkd->bk", feat.astype(np.float64),
                                 mem[pos_cols].astype(np.float64))
    cross = lse_full - x_pos.mean(axis=1)

    # ---- online loss ----
    # v8[k, b, w] = fp8-accurate max of sims' over window w of core/camera k
    # (proxies cam_prox[k, w*64 .. +64]).  Select candidate windows per row:
    # the global top windows (covers the reference's top-(BG_KNN+POSK)
    # proxies: the window holding the k-th largest value always ranks within
    # the top-k windows) plus every window within DELTA of its camera's best
    # (covers per-camera argmax).  Expand the selected windows and recompute
    # exact fp32 sims'/x there.  Margins sized for fp8 matmul noise
    # (sigma ~0.05 on window maxes).
    W = NCORES * NWIN_CORE                                    # 512 windows/row
    wv = np.moveaxis(v8, 0, 1).reshape(B, W)                  # [B, 512] k-major
    cam_of_w = np.repeat(np.arange(C), NWIN_CORE)             # [512]
    DELTA = 0.4
    JG = 88                                                   # global windows
    cammax = wv.reshape(B, C, NWIN_CORE).max(axis=2)          # [B, C]
    boost = wv >= (cammax[:, cam_of_w] - DELTA)               # near-camera-top
    nboost = int(boost.sum(axis=1).max())
    J = JG + max(nboost, C)
    prio = wv + 1e9 * boost
    sel_w = np.argpartition(-prio, J - 1, axis=1)[:, :J]      # [B, J] unique

    k_of = sel_w // NWIN_CORE                                 # camera/core
    w_of = sel_w % NWIN_CORE
    pid = cam_prox[k_of[:, :, None],
                   (w_of * WIN)[:, :, None] + np.arange(WIN)[None, None, :]]
    pid_b = pid.reshape(B, J * WIN)
    cam_of_cand = np.repeat(cam_of_w[sel_w], WIN, axis=1)     # [B, J*WIN]

    # exact fp32 recompute at the candidate proxies (row-chunked: the
    # gather is the memory hog)
    NCAND = J * WIN
    s_cand = np.empty((B, NCAND), np.float32)
    q_cand = np.empty((B, NCAND), np.float32)
    for lo in range(0, B, 32):
        hi = lo + 32
        memg = mem[pid_b[lo:hi]]                              # [32, NCAND, D]
        s_cand[lo:hi] = np.einsum("bd,bjd->bj", feat[lo:hi], memg)
        q_cand[lo:hi] = np.einsum("bd,bjd->bj", memprx[lo:hi], memg)
    simsp = s_cand.astype(np.float64) + RATIO * q_cand.astype(np.float64)
    x_cand = INV_TEMP * s_cand.astype(np.float64)

    # per-camera global argmax over candidates (exact values)
    tops_val = np.full((B, C), -np.inf)
    tops_j = np.zeros((B, C), np.int64)
    for c in range(C):
        sub = np.where(cam_of_cand == c, simsp, -np.inf)
        a = sub.argmax(axis=1)
        tops_j[:, c] = a
        tops_val[:, c] = sub[rows, a]

    # top-3 cameras by their best sims'
    order = np.argsort(-tops_val, axis=1)[:, :POSK]           # [B, 3]
    chosen_j = np.take_along_axis(tops_j, order, axis=1)      # [B, 3] cand idx
    chosen_pid = np.take_along_axis(pid_b, chosen_j, axis=1)  # [B, 3]

    # top-50 of the remaining candidates (windows are disjoint, so every
    # candidate proxy appears once; only the chosen need masking)
    is_chosen = (pid_b[:, :, None] == chosen_pid[:, None, :]).any(axis=2)
    Vmask = np.where(is_chosen, -np.inf, simsp)
    sel_idx = np.argpartition(-Vmask, BG_KNN, axis=1)[:, :BG_KNN]     # [B, 50]

    x_chosen = np.take_along_axis(x_cand, chosen_j, axis=1)   # [B, 3]
    x_sel = np.take_along_axis(x_cand, sel_idx, axis=1)       # [B, 50]
    xA = np.concatenate([x_chosen, x_sel], axis=1)            # [B, 53]
    mA = xA.max(axis=1)
    lse3 = mA + np.log(np.exp(xA - mA[:, None]).sum(axis=1))
    online = lse3 - x_chosen.mean(axis=1)

    # ---- camera-mean-sum ----
    dbg = globals().get("_DEBUG_COMPS")
    if dbg is not None:
        dbg["intra"] = intra.copy()
        dbg["cross"] = cross.copy()
        dbg["online"] = online.copy()
    total = 0.0
    for c in range(C):
        m = cams_h == c
        if m.any():
            total += intra[m].mean() + cross[m].mean() + online[m].mean()
    return np.float32(total)
